# revision 25
# baseline (speedup 1.0000x reference)
"""OTTT fused Dense+LIF spike step on 8 trn2 NeuronCores.

out = ((x @ W + b + 0.5*u0) >= 1.0).astype(f32)   x:[2048,4096] W:[4096,4096]

Sharding: 2x4 grid — batch split in 2 (1024 rows), W columns split in 4
(1024 cols). Per-core HBM traffic is 38MB (vs 76MB for pure data-parallel).

Matmul runs as a SINGLE float32r pass: the PE reads 4-byte fp32 and
truncates to FP22 (e10m11) at full bf16 rate (1 col/cycle), so no on-chip
hi/lo splitting is needed. x and W are pre-rounded to fp22 on the host
(round-to-nearest-even on the top 11 mantissa bits) so the hardware
truncation is exact and unbiased; the residual (dropped x_lo/W_lo cross
terms, ~2^-12 u-noise) flips only a few hundred spikes out of 8.4M,
comfortably under the 2e-2 rel-err gate. x is also pre-transposed on the
host so the kernel needs no PE transposes and PSUM is free for 8
accumulator banks.
"""

import os

import numpy as np

B = 2048
D = 4096
NCORES = 8
RS = 2            # batch split
CS = 4            # column split
BC = B // RS      # rows per core   (1024)
DC = D // CS      # cols per core   (1024)

LAST_RESULTS = None
_NC_CACHE = {}


def round22(a):
    """Round fp32 array to fp22 (e10m11): RNE on the top 11 mantissa bits."""
    u = a.view(np.uint32)
    lsb = (u >> 12) & np.uint32(1)
    u = u + np.uint32(0x7FF) + lsb
    u &= np.uint32(0xFFFFF000)
    return u.view(np.float32)


def build_nc(bc=BC, dc=DC, d=D, n_tile=512, reps=1):
    """Per-core bass program (SPMD: every core runs this).

    Inputs (per core): xt [d, bc] f32 (x-shard transposed, fp22-rounded),
    w [NT, d, n_tile] f32 (W col-shard, n-major, fp22-rounded), b [dc] f32,
    u0 [bc, dc] bf16. Output: out [bc, dc] f32 (0/1 spikes).
    """
    import concourse.bass as bass
    import concourse.mybir as mybir
    import concourse.tile as tile
    from concourse import bacc
    from concourse.alu_op_type import AluOpType

    f32 = mybir.dt.float32
    f32r = mybir.dt.float32r
    bf16 = mybir.dt.bfloat16
    u8 = mybir.dt.uint8
    P = 128
    MT = bc // P          # 8 m-tiles
    KT = d // P           # 32 k-tiles
    NT = dc // n_tile     # 2 n-groups
    XKG = 4               # k-tiles per steady-state xt DMA chunk (2MB)
    WKG = 4               # k-tiles per W DMA chunk (1MB)

    nc = bacc.Bacc(None, target_bir_lowering=False)
    xt = nc.dram_tensor("xt", [d, bc], f32r, kind="ExternalInput")
    w = nc.dram_tensor("w", [NT, d, n_tile], f32r, kind="ExternalInput")
    bvec = nc.dram_tensor("b", [dc], f32, kind="ExternalInput")
    u0 = nc.dram_tensor("u0", [bc, dc], bf16, kind="ExternalInput")
    out = nc.dram_tensor("out", [bc, dc], u8, kind="ExternalOutput")

    # DMA chunk plans (in k-tiles). The first chunks are small so the very
    # first matmul's operands arrive within a few us; later chunks are
    # ~1MB for full DMA efficiency. All input loads share the sync HWDGE
    # ring IN ORDER (SDMA round-robins across queues, so ring position is
    # the only way to sequence transfers): xt/W(n=0) interleaved by k,
    # then W(n=1) chunk 0, then u0, then the rest of W(n=1).
    xt_plan = [1, 1, 2] + [XKG] * ((KT - 4) // XKG)
    w0_plan = [1, 1, 2, 4] + [2 * WKG] * ((KT - 8) // (2 * WKG))
    w1_plan = [2 * WKG] * (KT // (2 * WKG))

    with tile.TileContext(nc) as tc:
        with (
            tc.tile_pool(name="const", bufs=1) as const,
            tc.tile_pool(name="xtp", bufs=1) as xtp,
            tc.tile_pool(name="wp1", bufs=2) as wp1,
            tc.tile_pool(name="wp2", bufs=1) as wp2,
            tc.tile_pool(name="wp4", bufs=1) as wp4,
            tc.tile_pool(name="wp8", bufs=3) as wp8,
            tc.tile_pool(name="up", bufs=1) as up,
            tc.tile_pool(name="op", bufs=3) as op,
            tc.tile_pool(name="psp", bufs=1, space="PSUM") as psp,
        ):
            thr = const.tile([P, dc], f32)
            idh = const.tile([P, P], bf16)

            for _rep in range(reps):
                # resident x^T chunks; xkt[ko] -> (tile, kt offset in chunk)
                xkt = {}

                def load_xt(kt0, nkt):
                    t = xtp.tile([P, nkt * bc], f32r, name=f"xt{kt0}")
                    ap = bass.AP(
                        xt, kt0 * P * bc, [[bc, P], [P * bc, nkt], [1, bc]]
                    )
                    nc.sync.dma_start(t[:], ap)
                    for i in range(nkt):
                        xkt[kt0 + i] = (t, i)

                def load_w(n, kt0, nkt):
                    wp = {1: wp1, 2: wp2, 4: wp4, 8: wp8}[nkt]
                    t = wp.tile([P, nkt * n_tile], f32r, name=f"wt{nkt}")
                    ap = bass.AP(
                        w,
                        n * d * n_tile + kt0 * P * n_tile,
                        [[n_tile, P], [P * n_tile, nkt], [1, n_tile]],
                    )
                    nc.sync.dma_start(t[:], ap)
                    return t

                def load_u(coff, eng):
                    # all MT m-tiles of one u0 column-half in one 1MB DMA.
                    # Shared buffer ("uu" x2 on a bufs=1 ring): the n=1
                    # half's DMA waits until pre0 has consumed the n=0 half.
                    t = up.tile([P, MT * n_tile], bf16, name="uu")
                    ap = bass.AP(
                        u0, coff, [[dc, P], [P * dc, MT], [1, n_tile]]
                    )
                    eng.dma_start(t[:], ap)
                    return t

                def mm_group(ps, wt, kt0, nkt, ut=None):
                    # last group (ut given): m-major, and each m's group is
                    # closed by ps[m] += 0.5*I @ u0[m] so the epilogue is a
                    # single is_ge against thr
                    last = ut is not None
                    its = (
                        [(kt, m) for m in range(MT) for kt in range(nkt)]
                        if last
                        else [(kt, m) for kt in range(nkt) for m in range(MT)]
                    )
                    for kt, m in its:
                        ko = kt0 + kt
                        xc, goff = xkt[ko]
                        nc.tensor.matmul(
                            ps[m][:],
                            xc[:, goff * bc + m * P:goff * bc + (m + 1) * P],
                            wt[:, kt * n_tile:(kt + 1) * n_tile],
                            start=(ko == 0),
                            stop=False,
                        )
                        if last and kt == nkt - 1:
                            nc.tensor.matmul(
                                ps[m][:],
                                idh[:],
                                ut[:, m * n_tile:(m + 1) * n_tile],
                                start=False,
                                stop=True,
                            )

                if _rep == 0:
                    nc.gpsimd.memset(idh[:], 0.0)
                    nc.gpsimd.affine_select(
                        out=idh[:], in_=idh[:],
                        compare_op=mybir.AluOpType.not_equal, fill=0.5,
                        base=0, pattern=[[-1, P]], channel_multiplier=1,
                    )


                # ---- n = 0: xt + W loads interleaved by k ----
                ps = [
                    psp.tile([P, n_tile], f32, name=f"ps{m}")
                    for m in range(MT)
                ]
                xt_cursor = 0  # next k-tile to issue an xt chunk for
                xt_i = 0
                kt0 = 0
                ngrp = len(w0_plan)
                for wi, nkt in enumerate(w0_plan):
                    while xt_cursor < kt0 + nkt:
                        load_xt(xt_cursor, xt_plan[xt_i])
                        xt_cursor += xt_plan[xt_i]
                        xt_i += 1
                    if wi == 3:
                        # thr and the u0 n=0-half ride the SWDGE (gpsimd)
                        # path mid-phase: off the startup-critical window,
                        # and early enough that pre0 is ready long before
                        # the n0->n1 PSUM-bank handoff.
                        if _rep == 0:
                            b_bcast = bass.AP(bvec, 0, [[0, P], [1, dc]])
                            nc.gpsimd.dma_start(out=thr[:], in_=b_bcast)
                            nc.vector.tensor_scalar(
                                out=thr[:], in0=thr[:], scalar1=-1.0,
                                scalar2=1.0,
                                op0=AluOpType.mult, op1=AluOpType.add,
                            )
                        ua = load_u(0, nc.gpsimd)
                    wt = load_w(0, kt0, nkt)
                    mm_group(
                        ps, wt, kt0, nkt,
                        ut=ua if wi == ngrp - 1 else None,
                    )
                    kt0 += nkt

                # W(n=1) chunk 0, then u0 n=1-half, then the rest of W(n=1)
                # — all behind the n=0 loads on the sync ring so they can't
                # steal startup bandwidth
                w1_tiles = [load_w(1, 0, w1_plan[0])]
                k1 = w1_plan[0]
                ub = None
                for wi, nkt in enumerate(w1_plan[1:]):
                    w1_tiles.append(load_w(1, k1, nkt))
                    k1 += nkt
                    if wi == 1:
                        ub = load_u(n_tile, nc.sync)

                # n = 0 epilogue: single is_ge per m frees the PSUM
                # bank for n = 1 almost immediately (0.5*u0 and b are
                # already inside ps/thr)
                for m in range(MT):
                    msl = slice(m * P, (m + 1) * P)
                    ot = op.tile([P, n_tile], u8)
                    nc.vector.tensor_tensor(
                        out=ot[:], in0=ps[m][:], in1=thr[:, 0:n_tile],
                        op=AluOpType.is_ge,
                    )
                    nc.scalar.dma_start(out[msl, 0:n_tile], ot[:])

                # ---- n = 1 matmuls ----
                ps = [
                    psp.tile([P, n_tile], f32, name=f"ps{m}")
                    for m in range(MT)
                ]
                kt0 = 0
                for wi, nkt in enumerate(w1_plan):
                    mm_group(
                        ps, w1_tiles[wi], kt0, nkt,
                        ut=ub if wi == len(w1_plan) - 1 else None,
                    )
                    kt0 += nkt

                # n = 1 epilogue: ps >= thr  (1 DVE op)
                nsl = slice(n_tile, dc)
                for m in range(MT):
                    msl = slice(m * P, (m + 1) * P)
                    ot = op.tile([P, n_tile], u8)
                    nc.vector.tensor_tensor(
                        out=ot[:], in0=ps[m][:], in1=thr[:, n_tile:dc],
                        op=AluOpType.is_ge,
                    )
                    nc.scalar.dma_start(out[msl, nsl], ot[:])

    nc.compile()
    return nc


def make_in_maps(x, W, b, u0):
    import ml_dtypes

    x = round22(np.ascontiguousarray(np.asarray(x, dtype=np.float32)))
    W = round22(np.ascontiguousarray(np.asarray(W, dtype=np.float32)))
    b = np.ascontiguousarray(np.asarray(b, dtype=np.float32))
    u0 = np.asarray(u0, dtype=np.float32)

    n_tile = 512
    NT = DC // n_tile
    xts = [np.ascontiguousarray(x[r * BC:(r + 1) * BC, :].T) for r in range(RS)]
    wns = [
        np.ascontiguousarray(
            W[:, c * DC:(c + 1) * DC]
            .reshape(D, NT, n_tile)
            .transpose(1, 0, 2)
        )
        for c in range(CS)
    ]
    bs = [np.ascontiguousarray(b[c * DC:(c + 1) * DC]) for c in range(CS)]
    u0s = [
        [
            np.ascontiguousarray(
                u0[r * BC:(r + 1) * BC, c * DC:(c + 1) * DC]
            ).astype(ml_dtypes.bfloat16)
            for c in range(CS)
        ]
        for r in range(RS)
    ]
    maps = []
    for core in range(NCORES):
        r, c = divmod(core, CS)
        maps.append(
            {"xt": xts[r], "w": wns[c], "b": bs[c], "u0": u0s[r][c]}
        )
    return maps


def kernel(x, W, b, u0, a_hat0=None, **_unused):
    global LAST_RESULTS
    from concourse.bass_utils import run_bass_kernel_spmd

    # Under axon, run_bass_kernel_spmd's trace path needs antenv.axon_hooks;
    # if this environment lacks it, force trace off rather than crash.
    try:
        from concourse._compat import axon_active

        if axon_active():
            import antenv.axon_hooks  # noqa: F401
    except ImportError:
        os.environ["BASS_NEVER_TRACE"] = "1"

    if "full" not in _NC_CACHE:
        _NC_CACHE["full"] = build_nc()
    nc = _NC_CACHE["full"]

    in_maps = make_in_maps(x, W, b, u0)
    res = run_bass_kernel_spmd(nc, in_maps, list(range(NCORES)))
    LAST_RESULTS = res
    full = np.empty((B, D), dtype=np.float32)
    for core in range(NCORES):
        r, c = divmod(core, CS)
        full[r * BC:(r + 1) * BC, c * DC:(c + 1) * DC] = res.results[core][
            "out"
        ].astype(np.float32)
    return full
